# revision 1
# baseline (speedup 1.0000x reference)
"""Trainium2 Bass kernel for the CDGR gnn_message_passing module.

Mathematically exact reformulation of the reference (verified to ~4e-4
scale-relative error vs the fp32 jax reference, dominated by float32r
fp22 matmul truncation):

  - softmax rows of A sum to 1  =>  L = I - A, the d-scaling vanishes
  - s2l logits are additively separable in (pixel, node) => the softmax
    over pixels is identical for every node column => app collapses to a
    rank-1 outer product relu(G) (x) softmax(w_in . x)
  - the semantic branch (word attention + 2-layer GCN) is batch
    independent => computed once per core
  - the two chained 1x1 convs fuse: Wlg = final_w[:, :C] @ gw_w

Per batch (2 per core, data-parallel over 8 cores):
  out[o,q] = relu( Wlg @ spiral^T + fa (x) sa + x )  with
  spiral = xv - (E @ xv) / D,  E = exp(S - ub),  S = x_phi @ Dg @ x_phi^T
  computed via S^T tiles (lhsT = M_ext columns) so that E^T column
  slices feed the big E @ xv matmul directly as lhsT, with a fused ones
  column in xv giving D, and a fused K=17 row giving the -ub shift.
"""

import os
from contextlib import ExitStack

import numpy as np

import concourse.bass as bass
import concourse.bacc as bacc
import concourse.mybir as mybir
import concourse.tile as tile
from concourse import masks
from concourse.bass_utils import run_bass_kernel_spmd

FP = mybir.dt.float32
FR = mybir.dt.float32r
AF = mybir.ActivationFunctionType
ALU = mybir.AluOpType

NCORES = 8
BPC = 2          # batches per core
C, HW = 256, 1024
MPHI, NN, DE = 16, 20, 300
KE = DE + 1      # 301 = DEMB + fused-bias row

LAST_EXEC_NS = None
LAST_RESULT = None


def _ksl(total, step=128):
    return [(o, min(step, total - o)) for o in range(0, total, step)]


def _fr(ap):
    return ap.bitcast(FR)


def _mm(nc, out, lhsT, rhs, start, stop):
    nc.tensor.matmul(out, lhsT.bitcast(FR), rhs.bitcast(FR), start=start, stop=stop)


def _build_nc(reps=1):
    nc = bacc.Bacc()

    def par(name, shape, out=False):
        return nc.declare_dram_parameter(name, list(shape), FP, isOutput=out)

    x_p = par("x", [BPC, C * HW])
    out_p = par("out", [BPC, C * HW], out=True)
    emb_p = par("emb", [NN, DE])
    embTe_p = par("embTe", [KE, NN])          # [emb.T ; ones]
    adj_p = par("adj", [NN, NN])
    wq_p = par("wq", [DE, DE])                # natural
    bq_p = par("bq_col", [DE, 1])
    wk_p = par("wk", [DE, DE])
    bk_p = par("bk_col", [DE, 1])
    wve_p = par("wve", [KE, DE])              # [wv ; bv]
    wo_p = par("wo", [DE, DE])
    bo_p = par("bo_row", [1, DE])
    phiwT_p = par("phi_wT", [C, MPHI])
    phib_p = par("phi_b_col", [MPHI, 1])
    globwT_p = par("glob_wT", [C, MPHI])
    gc1_p = par("gc1_w", [DE, C])
    gc2_p = par("gc2_w", [C, C])
    gww_p = par("gw_w", [C, C])
    win_p = par("w_in_col", [C, 1])
    fwT_p = par("final_wT", [2 * C, C])
    ccol_p = par("const_col", [128, 16])
    crow_p = par("const_row", [1, HW])

    rscr = nc.dram_tensor("rscratch", [reps * BPC, MPHI * HW], FP)

    with tile.TileContext(nc) as tc:
        with nc.allow_low_precision(reason="float32r matmul feed tags"), \
             ExitStack() as ctx:
            _body(ctx, tc, nc, dict(
                x=x_p, out=out_p, emb=emb_p, embTe=embTe_p, adj=adj_p,
                wq=wq_p, bq=bq_p, wk=wk_p, bk=bk_p, wve=wve_p, wo=wo_p, bo=bo_p,
                phiwT=phiwT_p, phib=phib_p, globwT=globwT_p,
                gc1=gc1_p, gc2=gc2_p, gww=gww_p, win=win_p, fwT=fwT_p,
                ccol=ccol_p, crow=crow_p,
                rscr=rscr,
            ), reps=reps)
    nc.finalize()
    return nc


def _body(ctx, tc, nc, P, reps=1):
    cw = ctx.enter_context(tc.tile_pool(name="cw", bufs=1))       # consts/weights
    sm = ctx.enter_context(tc.tile_pool(name="sm", bufs=2))       # small working
    med = ctx.enter_context(tc.tile_pool(name="med", bufs=2))     # wide, 1 per batch
    big = ctx.enter_context(tc.tile_pool(name="big", bufs=4))     # [128,1024]-ish
    etp = ctx.enter_context(tc.tile_pool(name="etp", bufs=10))    # E^T tiles
    xvp = ctx.enter_context(tc.tile_pool(name="xvp", bufs=14))    # xv_ext tiles
    ps_w = ctx.enter_context(tc.tile_pool(name="ps_w", bufs=2, space="PSUM"))
    ps_x = ctx.enter_context(tc.tile_pool(name="ps_x", bufs=2, space="PSUM"))
    ps_t = ctx.enter_context(tc.tile_pool(name="ps_t", bufs=2, space="PSUM"))

    def load_w(dram, k, n, tag, fr=True):
        """DRAM [k, n] -> list of SBUF tiles [<=128, n] along k."""
        ts = []
        for i, (o, s) in enumerate(_ksl(k)):
            t = cw.tile([s, n], FP, tag=f"{tag}{i}")
            if fr:
                nc.sync.dma_start(_fr(t[:]), _fr(dram[o:o + s, :]))
            else:
                nc.sync.dma_start(t[:], dram[o:o + s, :])
            ts.append(t)
        return ts

    ident = cw.tile([128, 128], FP, tag="ident")
    masks.make_identity(nc, ident[:])

    embTe = load_w(P["embTe"][:], KE, NN, "embTe")
    emb_sb = load_w(P["emb"][:], NN, DE, "emb", fr=False)[0]
    adj_sb = load_w(P["adj"][:], NN, NN, "adj", fr=False)[0]
    wq = load_w(P["wq"][:], DE, DE, "wq")
    wk = load_w(P["wk"][:], DE, DE, "wk")
    bq = load_w(P["bq"][:], DE, 1, "bq", fr=False)
    bk = load_w(P["bk"][:], DE, 1, "bk", fr=False)
    wve = load_w(P["wve"][:], KE, DE, "wve")
    wo = load_w(P["wo"][:], DE, DE, "wo")
    bo_row = load_w(P["bo"][:], 1, DE, "bo_row", fr=False)[0]
    phiwT = load_w(P["phiwT"][:], C, MPHI, "phiwT")
    phib = load_w(P["phib"][:], MPHI, 1, "phib", fr=False)[0]
    globwT = load_w(P["globwT"][:], C, MPHI, "globwT")
    gc1 = load_w(P["gc1"][:], DE, C, "gc1")
    gc2 = load_w(P["gc2"][:], C, C, "gc2")
    gww = load_w(P["gww"][:], C, C, "gww")
    win = load_w(P["win"][:], C, 1, "win")
    fwT = load_w(P["fwT"][:], 2 * C, C, "fwT")

    one_row = cw.tile([1, NN], FP, tag="one_row")
    nc.sync.dma_start(_fr(one_row[:]), _fr(P["crow"][0:1, 0:NN]))
    ones20 = cw.tile([NN, 8], FP, tag="ones20")
    nc.sync.dma_start(_fr(ones20[:]), _fr(P["ccol"][0:NN, 0:8]))
    inv20 = cw.tile([NN, 8], FP, tag="inv20")
    nc.sync.dma_start(_fr(inv20[:]), _fr(P["ccol"][0:NN, 8:16]))
    onescol = cw.tile([128, 8], FP, tag="onescol")
    nc.sync.dma_start(onescol[:], P["ccol"][:, 0:8])

    # ---------------- semantic branch (batch independent) ----------------
    # qT, kT [300, 20] in 3 partition chunks: qT = wq^T @ emb^T (+ bias col)
    def qt_like(w, bcol, tag):
        outs = []
        for mi, (mo, ms) in enumerate(_ksl(DE)):
            ps = ps_t.tile([ms, NN], FP, tag="ps_t")
            for ki, (ko, ks) in enumerate(_ksl(DE)):
                _mm(nc, ps[:], w[ki][:, mo:mo + ms], embTe[ki][0:ks, :],
                    start=(ki == 0), stop=(ki == 2))
            t = sm.tile([ms, NN], FP, tag=f"{tag}{mi}")
            nc.scalar.activation(_fr(t[:]), ps[:], AF.Identity, bias=bcol[mi][:, 0:1])
            outs.append(t)
        return outs

    qT = qt_like(wq, [bq[0], bq[1], bq[2]], "qT")
    kT = qt_like(wk, [bk[0], bk[1], bk[2]], "kT")

    # v natural [20, 300] = embTe.T @ wve (bias row fused)
    ps = ps_t.tile([NN, DE], FP, tag="ps_t")
    for ki, (ko, ks) in enumerate(_ksl(KE)):
        _mm(nc, ps[:], embTe[ki][:, :], wve[ki][:, :], start=(ki == 0), stop=(ki == 2))
    v_sb = sm.tile([NN, DE], FP, tag="v_sb")
    nc.scalar.copy(_fr(v_sb[:]), ps[:])

    # att = softmax(q @ k.T / sqrt(300)) : [20, 20]
    ps = ps_t.tile([NN, NN], FP, tag="ps_t")
    for ki, (ko, ks) in enumerate(_ksl(DE)):
        _mm(nc, ps[:], qT[ki][:, :], kT[ki][:, :], start=(ki == 0), stop=(ki == 2))
    att_s = sm.tile([NN, NN], FP, tag="att_s")
    nc.scalar.activation(att_s[:], ps[:], AF.Identity, scale=float(1.0 / np.sqrt(DE)))
    mx = sm.tile([NN, 1], FP, tag="mx")
    nc.vector.tensor_reduce(mx[:], att_s[:], axis=mybir.AxisListType.X, op=ALU.max)
    negmx = sm.tile([NN, 1], FP, tag="negmx")
    nc.vector.tensor_scalar_mul(negmx[:], mx[:], -1.0)
    att_e = sm.tile([NN, NN], FP, tag="att_e")
    rs = sm.tile([NN, 1], FP, tag="rs")
    nc.scalar.activation(att_e[:], att_s[:], AF.Exp, bias=negmx[:, 0:1], accum_out=rs[:, 0:1])
    rr = sm.tile([NN, 1], FP, tag="rr")
    nc.vector.reciprocal(rr[:], rs[:])
    att_n = sm.tile([NN, NN], FP, tag="att_n")
    nc.vector.tensor_scalar_mul(att_n[:], att_e[:], rr[:, 0:1])

    # attT, AV = att @ v, node1col = AV^T @ (1/20)
    ps = ps_t.tile([NN, NN], FP, tag="ps_t")
    nc.tensor.transpose(ps[:], att_n[:], ident[0:NN, 0:NN])
    attT = sm.tile([NN, NN], FP, tag="attT")
    nc.scalar.copy(_fr(attT[:]), ps[:])
    ps = ps_t.tile([NN, DE], FP, tag="ps_t")
    _mm(nc, ps[:], attT[:, :], v_sb[:, :], start=True, stop=True)
    av_sb = sm.tile([NN, DE], FP, tag="av_sb")
    nc.scalar.copy(_fr(av_sb[:]), ps[:])

    n1c = sm.tile([128, 3], FP, tag="n1c")
    for mi, (mo, ms) in enumerate(_ksl(DE)):
        ps = ps_t.tile([ms, 8], FP, tag="ps_t")
        _mm(nc, ps[:], av_sb[:, mo:mo + ms], inv20[:, :], start=True, stop=True)
        nc.scalar.copy(_fr(n1c[0:ms, mi:mi + 1]), ps[:, 0:1])

    # node2 [1,300] = node1^T @ wo + bo ; ev = emb + bcast(node2)
    ps = ps_t.tile([1, DE], FP, tag="ps_t")
    for ki, (ko, ks) in enumerate(_ksl(DE)):
        _mm(nc, ps[:], n1c[0:ks, ki:ki + 1], wo[ki][:, :],
            start=(ki == 0), stop=(ki == 2))
    n2 = sm.tile([1, DE], FP, tag="n2")
    nc.vector.tensor_add(_fr(n2[:]), bo_row[:], ps[:])
    ps = ps_t.tile([NN, DE], FP, tag="ps_t")
    _mm(nc, ps[:], one_row[:, :], n2[:, :], start=True, stop=True)
    ev_sb = sm.tile([NN, DE], FP, tag="ev_sb")
    nc.vector.tensor_add(ev_sb[:], emb_sb[:], ps[:])

    # evT chunks [<=128, 20]
    evT = []
    for mi, (mo, ms) in enumerate(_ksl(DE)):
        ps = ps_t.tile([ms, NN], FP, tag="ps_t")
        nc.tensor.transpose(ps[:], ev_sb[:, mo:mo + ms], ident[0:NN, 0:NN])
        t = sm.tile([ms, NN], FP, tag=f"evT{mi}")
        nc.scalar.copy(_fr(t[:]), ps[:])
        evT.append(t)

    # adj_n = (d (x) d) * (adj + I)
    ah = sm.tile([NN, NN], FP, tag="ah")
    nc.gpsimd.tensor_add(ah[:], adj_sb[:], ident[0:NN, 0:NN])
    r20 = sm.tile([NN, 1], FP, tag="r20")
    nc.vector.tensor_reduce(r20[:], ah[:], axis=mybir.AxisListType.X, op=ALU.add)
    ir20 = sm.tile([NN, 1], FP, tag="ir20")
    nc.vector.reciprocal(ir20[:], r20[:])
    d20 = sm.tile([NN, 1], FP, tag="d20")
    nc.scalar.activation(d20[:], ir20[:], AF.Sqrt)
    ps = ps_t.tile([1, NN], FP, tag="ps_t")
    nc.tensor.transpose(ps[:], d20[:, 0:1], ident[0:NN, 0:NN])
    dT = sm.tile([1, NN], FP, tag="dT")
    nc.scalar.copy(_fr(dT[:]), ps[:])
    ps = ps_t.tile([NN, NN], FP, tag="ps_t")
    _mm(nc, ps[:], dT[:, :], dT[:, :], start=True, stop=True)
    adjn = sm.tile([NN, NN], FP, tag="adjn")
    nc.vector.tensor_mul(adjn[:], ah[:], ps[:])
    ps = ps_t.tile([NN, NN], FP, tag="ps_t")
    nc.tensor.transpose(ps[:], adjn[:], ident[0:NN, 0:NN])
    adjnT = sm.tile([NN, NN], FP, tag="adjnT")
    nc.scalar.copy(_fr(adjnT[:]), ps[:])

    # GCN layer 1: g1 = relu(adj_n @ (ev @ gc1_w))
    ps = ps_t.tile([NN, C], FP, tag="ps_t")
    for ki in range(3):
        _mm(nc, ps[:], evT[ki][:, :], gc1[ki][:, :], start=(ki == 0), stop=(ki == 2))
    t1 = sm.tile([NN, C], FP, tag="t1")
    nc.scalar.copy(_fr(t1[:]), ps[:])
    ps = ps_t.tile([NN, C], FP, tag="ps_t")
    _mm(nc, ps[:], adjnT[:, :], t1[:, :], start=True, stop=True)
    g1 = sm.tile([NN, C], FP, tag="g1")
    nc.scalar.activation(g1[:], ps[:], AF.Relu)

    g1T = []
    for mi, (mo, ms) in enumerate(_ksl(C)):
        ps = ps_t.tile([ms, NN], FP, tag="ps_t")
        nc.tensor.transpose(ps[:], g1[:, mo:mo + ms], ident[0:NN, 0:NN])
        t = sm.tile([ms, NN], FP, tag=f"g1T{mi}")
        nc.scalar.copy(_fr(t[:]), ps[:])
        g1T.append(t)

    ps = ps_t.tile([NN, C], FP, tag="ps_t")
    for ki in range(2):
        _mm(nc, ps[:], g1T[ki][:, :], gc2[ki][:, :], start=(ki == 0), stop=(ki == 1))
    t2 = sm.tile([NN, C], FP, tag="t2")
    nc.scalar.copy(_fr(t2[:]), ps[:])
    ps = ps_t.tile([NN, C], FP, tag="ps_t")
    _mm(nc, ps[:], adjnT[:, :], t2[:, :], start=True, stop=True)
    g2 = sm.tile([NN, C], FP, tag="g2")
    nc.scalar.activation(_fr(g2[:]), ps[:], AF.Relu)

    # reluG [128, 2] (column cb = relu(sum_m g2[m, 128cb:128cb+128]))
    reluG = sm.tile([128, 2], FP, tag="reluG")
    for cb in range(2):
        ps = ps_t.tile([128, 8], FP, tag="ps_t")
        _mm(nc, ps[:], g2[:, 128 * cb:128 * (cb + 1)], ones20[:, :], start=True, stop=True)
        nc.scalar.activation(_fr(reluG[:, cb:cb + 1]), ps[:, 0:1], AF.Relu)

    # fa [1, 256] = reluG^T @ Wa^T  (Wa^T = final_wT rows 256:512)
    ps = ps_t.tile([1, C], FP, tag="ps_t")
    for cb in range(2):
        _mm(nc, ps[:], reluG[:, cb:cb + 1], fwT[2 + cb][:, :],
            start=(cb == 0), stop=(cb == 1))
    fa = sm.tile([1, C], FP, tag="fa")
    nc.scalar.copy(_fr(fa[:]), ps[:])

    # WlgT [256, 256] = gw_w^T-contracted:  WlgT[c,o] = sum_k gw_w[k,c] Wl^T[k,o]
    WlgT = []
    for cb in range(2):
        ps = ps_t.tile([128, C], FP, tag="ps_t")
        for ki in range(2):
            _mm(nc, ps[:], gww[ki][:, 128 * cb:128 * (cb + 1)], fwT[ki][:, :],
                start=(ki == 0), stop=(ki == 1))
        t = sm.tile([128, C], FP, tag=f"WlgT{cb}")
        nc.scalar.copy(_fr(t[:]), ps[:])
        WlgT.append(t)

    # ---------------- per-batch pipeline ----------------
    x_cq = P["x"][:].rearrange("b (c q) -> b c q", c=C)       # [b, 256, 1024]
    x_pc = P["x"][:].rearrange("b (p c) -> b p c", c=C)       # [b, 1024, 256]
    out_cq = P["out"][:].rearrange("b (c q) -> b c q", c=C)
    r_jq = P["rscr"][:].rearrange("b (j q) -> b j q", j=MPHI)
    r_pj = P["rscr"][:].rearrange("b (p j) -> b p j", j=MPHI)

    for rep in range(reps):
      for b in range(BPC):
        rs = rep * BPC + b
        # loads
        xmat = []
        for j in range(2):
            t = big.tile([128, HW], FP, tag="xmat")
            nc.sync.dma_start(_fr(t[:]), _fr(x_cq[b, 128 * j:128 * (j + 1), :]))
            xmat.append(t)
        xv = []
        for t8 in range(8):
            t = xvp.tile([128, C + 8], FP, tag="xv")
            nc.sync.dma_start(_fr(t[:, 0:C]), _fr(x_pc[b, 128 * t8:128 * (t8 + 1), :]))
            nc.scalar.copy(_fr(t[:, C:C + 8]), onescol[:, 0:8])
            xv.append(t)

        # phi = phi_w @ xmat + phi_b ; R = relu(phi)
        ps_phi = ps_w.tile([MPHI, HW], FP, tag="ps_w")
        for ki in range(2):
            for nh in range(2):
                _mm(nc, ps_phi[:, 512 * nh:512 * (nh + 1)],
                    phiwT[ki][:, :], xmat[ki][:, 512 * nh:512 * (nh + 1)],
                    start=(ki == 0), stop=(ki == 1))
        R = med.tile([MPHI, HW], FP, tag="R")
        nc.scalar.activation(_fr(R[:]), ps_phi[:], AF.Relu, bias=phib[:, 0:1])
        nc.sync.dma_start(r_jq[rs], R[:])

        # x_phi tiles [128,16] from scratch, PE-transpose into x_phiT_ext [17, 1024]
        xpT = med.tile([MPHI + 1, HW], FP, tag="xpT")
        for t8 in range(8):
            xp = sm.tile([128, MPHI], FP, tag="xp")
            nc.sync.dma_start(xp[:], r_pj[rs, 128 * t8:128 * (t8 + 1), :])
            ps = ps_t.tile([MPHI, 128], FP, tag="ps_t")
            nc.tensor.transpose(ps[:], xp[:], ident[:, :])
            nc.vector.tensor_copy(_fr(xpT[0:MPHI, 128 * t8:128 * (t8 + 1)]), ps[:])

        # g = glob_w @ mean(x) ; Dg entries
        xmean = sm.tile([128, 16], FP, tag="xmean")
        nc.vector.memset(xmean[:], 0.0)
        for ki in range(2):
            nc.vector.tensor_reduce(_fr(xmean[:, 8 * ki:8 * ki + 1]), xmat[ki][:],
                                    axis=mybir.AxisListType.X, op=ALU.add)
        ps_g = ps_t.tile([MPHI, 8], FP, tag="ps_t")
        for ki in range(2):
            _mm(nc, ps_g[:], globwT[ki][:, :], xmean[:, 8 * ki:8 * ki + 8],
                start=(ki == 0), stop=(ki == 1))
        sgm = sm.tile([MPHI, 1], FP, tag="sgm")
        nc.scalar.activation(sgm[:], ps_g[:, 0:1], AF.Sigmoid, scale=float(1.0 / HW))
        sm05 = sm.tile([MPHI, 1], FP, tag="sm05")
        nc.vector.tensor_scalar_add(sm05[:], sgm[:], -0.5)
        Dg = sm.tile([MPHI, MPHI], FP, tag="Dg")
        nc.vector.tensor_scalar(_fr(Dg[:]), ident[0:MPHI, 0:MPHI], sm05[:, 0:1], 0.5,
                                op0=ALU.mult, op1=ALU.add)

        # M_ext [17, 1024]: rows 0:16 = Dg @ R, row 16 = ones
        ps_m = ps_w.tile([MPHI, HW], FP, tag="ps_w")
        for nh in range(2):
            _mm(nc, ps_m[:, 512 * nh:512 * (nh + 1)], Dg[:, :],
                R[:, 512 * nh:512 * (nh + 1)], start=True, stop=True)
        Me = med.tile([MPHI + 1, HW], FP, tag="Me")
        nc.vector.tensor_copy(_fr(Me[0:MPHI, :]), ps_m[:])
        nc.sync.dma_start(_fr(Me[MPHI:MPHI + 1, :]), _fr(P["crow"][0:1, :]))

        # -ub row: negMmax = -max_q M ; xpT row 16 = negMmax^T @ xpT[0:16]
        Mmax = sm.tile([MPHI, 1], FP, tag="Mmax")
        nc.vector.tensor_reduce(Mmax[:], Me[0:MPHI, :], axis=mybir.AxisListType.X,
                                op=ALU.max)
        negMm = sm.tile([MPHI, 1], FP, tag="negMm")
        nc.vector.tensor_scalar_mul(_fr(negMm[:]), Mmax[:], -1.0)
        ps_ub = ps_w.tile([1, HW], FP, tag="ps_w")
        for nh in range(2):
            _mm(nc, ps_ub[:, 512 * nh:512 * (nh + 1)], negMm[:, 0:1],
                xpT[0:MPHI, 512 * nh:512 * (nh + 1)], start=True, stop=True)
        nub = sm.tile([1, HW], FP, tag="nub")
        nc.scalar.copy(_fr(nub[:]), ps_ub[:])
        nc.sync.dma_start(_fr(xpT[MPHI:MPHI + 1, :]), _fr(nub[:]))

        # S^T tiles + exp -> E^T tiles [128, 1024]
        ET = []
        for t8 in range(8):
            ps_st = ps_w.tile([128, HW], FP, tag="ps_w")
            for nh in range(2):
                _mm(nc, ps_st[:, 512 * nh:512 * (nh + 1)],
                    Me[:, 128 * t8:128 * (t8 + 1)],
                    xpT[:, 512 * nh:512 * (nh + 1)], start=True, stop=True)
            et = etp.tile([128, HW], FP, tag="et")
            nc.scalar.activation(_fr(et[:]), ps_st[:], AF.Exp)
            ET.append(et)

        # per p-tile: EXV = E @ xv_ext (col 256 = D); spiral; transpose
        spT = [big.tile([128, HW], FP, tag="spT", name=f"spT{b}_{i}")
               for i in range(2)]
        for pt in range(8):
            ps_e = ps_x.tile([128, C + 8], FP, tag="ps_x")
            for k in range(8):
                _mm(nc, ps_e[:], ET[k][:, 128 * pt:128 * (pt + 1)], xv[k][:, :],
                    start=(k == 0), stop=(k == 7))
            negD = sm.tile([128, 1], FP, tag="negD")
            nc.vector.tensor_scalar_mul(negD[:], ps_e[:, C:C + 1], -1.0)
            nrd = sm.tile([128, 1], FP, tag="nrd")
            nc.vector.reciprocal(nrd[:], negD[:])
            spr = sm.tile([128, C], FP, tag="spr")
            nc.vector.scalar_tensor_tensor(spr[:], ps_e[:, 0:C], nrd[:, 0:1],
                                           xv[pt][:, 0:C], op0=ALU.mult, op1=ALU.add)
            for ch in range(2):
                ps = ps_t.tile([128, 128], FP, tag="ps_t")
                nc.tensor.transpose(ps[:], spr[:, 128 * ch:128 * (ch + 1)], ident[:, :])
                if ch == 0:
                    nc.scalar.copy(_fr(spT[ch][:, 128 * pt:128 * (pt + 1)]), ps[:])
                else:
                    nc.vector.tensor_copy(_fr(spT[ch][:, 128 * pt:128 * (pt + 1)]), ps[:])

        # sa = softmax over pixels of w_in . x
        ps_a = ps_w.tile([1, HW], FP, tag="ps_w")
        for ki in range(2):
            for nh in range(2):
                _mm(nc, ps_a[:, 512 * nh:512 * (nh + 1)], win[ki][:, :],
                    xmat[ki][:, 512 * nh:512 * (nh + 1)],
                    start=(ki == 0), stop=(ki == 1))
        ea = med.tile([1, HW], FP, tag="ea")
        sae = sm.tile([1, 1], FP, tag="sae")
        nc.scalar.activation(ea[:], ps_a[:], AF.Exp, accum_out=sae[:, 0:1])
        sar = sm.tile([1, 1], FP, tag="sar")
        nc.vector.reciprocal(sar[:], sae[:])
        sa = med.tile([1, HW], FP, tag="sa")
        nc.vector.tensor_scalar_mul(_fr(sa[:]), ea[:], sar[:, 0:1])

        # out[o,:] = relu(Wlg @ spiral^T + fa (x) sa + x)
        for ot in range(2):
            ps_o = ps_w.tile([128, HW], FP, tag="ps_w")
            for nh in range(2):
                sl = slice(512 * nh, 512 * (nh + 1))
                for ct in range(2):
                    _mm(nc, ps_o[:, sl], WlgT[ct][:, 128 * ot:128 * (ot + 1)],
                        spT[ct][:, sl], start=(ct == 0), stop=False)
                _mm(nc, ps_o[:, sl], fa[0:1, 128 * ot:128 * (ot + 1)], sa[0:1, sl],
                    start=False, stop=True)
            ob = big.tile([128, HW], FP, tag="ob", bufs=2)
            for nh in range(2):
                sl = slice(512 * nh, 512 * (nh + 1))
                nc.vector.scalar_tensor_tensor(ob[:, sl], ps_o[:, sl], 1.0,
                                               xmat[ot][:, sl],
                                               op0=ALU.mult, op1=ALU.add)
                nc.scalar.activation(ob[:, sl], ob[:, sl], AF.Relu)
                nc.sync.dma_start(out_cq[b, 128 * ot:128 * (ot + 1), sl], ob[:, sl])


def _const_col():
    cc = np.zeros((128, 16), np.float32)
    cc[:, 0] = 1.0
    cc[:, 8] = 1.0 / NN
    return cc


def _prep_shared(inputs):
    f = lambda k: np.ascontiguousarray(inputs[k], dtype=np.float32)
    shared = {
        "emb": f("emb"),
        "embTe": np.ascontiguousarray(
            np.vstack([f("emb").T, np.ones((1, NN), np.float32)])),
        "adj": f("adj"),
        "wq": f("wq"), "bq_col": f("bq").reshape(DE, 1),
        "wk": f("wk"), "bk_col": f("bk").reshape(DE, 1),
        "wve": np.ascontiguousarray(np.vstack([f("wv"), f("bv")[None, :]])),
        "wo": f("wo"), "bo_row": f("bo").reshape(1, DE),
        "phi_wT": np.ascontiguousarray(f("phi_w").T),
        "phi_b_col": f("phi_b").reshape(MPHI, 1),
        "glob_wT": np.ascontiguousarray(f("glob_w").T),
        "gc1_w": f("gc1_w"), "gc2_w": f("gc2_w"), "gw_w": f("gw_w"),
        "w_in_col": f("s2l_w")[:C].reshape(C, 1).copy(),
        "final_wT": np.ascontiguousarray(f("final_w").T),
        "const_col": _const_col(),
        "const_row": np.ones((1, HW), np.float32),
    }
    return shared


_NC_CACHE = {}


def kernel(**inputs):
    global LAST_EXEC_NS, LAST_RESULT
    if "nc" not in _NC_CACHE:
        _NC_CACHE["nc"] = _build_nc()
    nc = _NC_CACHE["nc"]

    x = np.ascontiguousarray(inputs["x"], dtype=np.float32)
    B = x.shape[0]
    shared = _prep_shared(inputs)
    in_maps = []
    for i in range(NCORES):
        m = dict(shared)
        m["x"] = np.ascontiguousarray(
            x[i * BPC:(i + 1) * BPC].reshape(BPC, C * HW))
        in_maps.append(m)

    trace = os.environ.get("KERNEL_TRACE", "0") == "1"
    res = run_bass_kernel_spmd(nc, in_maps, list(range(NCORES)), trace=trace)
    LAST_RESULT = res
    LAST_EXEC_NS = getattr(res, "exec_time_ns", None)

    out = np.empty((B, C, 32, 32), np.float32)
    for i in range(NCORES):
        out[i * BPC:(i + 1) * BPC] = res.results[i]["out"].reshape(BPC, C, 32, 32)
    return out



# revision 8
# speedup vs baseline: 1.4243x; 1.4243x over previous
"""Trainium2 Bass kernel for the CDGR gnn_message_passing module.

Mathematically exact reformulation of the reference (see derivation in the
docstrings below):

  - softmax rows of A sum to 1  =>  L = I - A, the d-scaling vanishes
  - s2l logits are additively separable in (pixel, node) => the softmax
    over pixels is identical for every node column => app collapses to a
    rank-1 outer product relu(G) (x) softmax(w_in . x)
  - the semantic branch (word attention + 2-layer GCN) is batch
    independent => computed once per core (in bf16; it only feeds the
    rank-1 app term and is well inside the 2e-2 tolerance)
  - the two chained 1x1 convs fuse: Wlg = final_w[:, :C] @ gw_w
  - the `+ x` residual is folded into the final matmul as an
    identity-weight accumulation (frees the vector engine)

Per batch (2 per core, data-parallel over 8 cores):
  out[o,q] = relu( Wlg @ spiral^T + fa (x) ea + x )  with
  spiral = xv - (E @ xv) / D,  E = exp(S - ub),  S = x_phi @ Dg @ x_phi_T
  computed via S^T tiles (lhsT = M_ext columns) so that E^T column
  slices feed the big E @ xv matmul directly as lhsT, with a fused ones
  column in xv giving D, and a fused K=17 row giving the -ub shift.

I/O strategy (the previous version spent 114us of SP-sequencer time on 96
small DMAs): all weights/constants are host-packed into two [128, N] DRAM
images (one fp32 "hot" pack, one bf16 "semantic" pack) loaded with one DMA
each, and each batch moves exactly 5 wide strided DMAs (x natural view,
x raw-reshape view, R spill, x_phi reload, output).
"""

import os
from contextlib import ExitStack

import numpy as np

import concourse.bass as bass
import concourse.bacc as bacc
import concourse.mybir as mybir
import concourse.tile as tile
from concourse import masks
from concourse.bass_utils import run_bass_kernel_spmd

FP = mybir.dt.float32
BF = mybir.dt.bfloat16
FR = mybir.dt.float32r
AF = mybir.ActivationFunctionType
ALU = mybir.AluOpType
AX = mybir.AxisListType

NCORES = 8
BPC = 2          # batches per core
C, HW = 256, 1024
MPHI, NN, DE = 16, 20, 300
KE = DE + 1      # 301 = DEMB + fused-bias row

LAST_EXEC_NS = None
LAST_RESULT = None


def _ksl(total, step=128):
    return [(o, min(step, total - o)) for o in range(0, total, step)]


def _fr(ap):
    return ap.bitcast(FR)


# ---------------------------------------------------------------------------
# weight-pack layouts (shared between host packing and kernel build)
# ---------------------------------------------------------------------------

class _PackAlloc:
    """First-fit strip allocator: blocks of equal width stack vertically in a
    128-row strip before opening a new column range."""

    def __init__(self):
        self.strips = []            # [col_off, width, used_rows]
        self.ncols = 0
        self.blocks = {}            # name -> (row, col, rows, cols)

    def add(self, name, rows, cols, stack=False):
        # PE matmul operands must sit at base partition 0 (they pair with
        # base-0 tiles); only non-matmul blocks may stack below other blocks.
        if stack:
            for s in self.strips:
                r = (s[2] + 31) // 32 * 32
                if s[1] == cols and r <= 64 and r + rows <= 128:
                    s[2] = r + rows
                    self.blocks[name] = (r, s[0], rows, cols)
                    return
        off = self.ncols
        self.ncols += cols
        self.strips.append([off, cols, rows])
        self.blocks[name] = (0, off, rows, cols)


def _mk_layout_h():
    a = _PackAlloc()
    for i in range(2):
        a.add(f"phiwT{i}", 128, MPHI)
    for i in range(2):
        a.add(f"globwT{i}", 128, MPHI)
    for i in range(2):
        a.add(f"win{i}", 128, 1)
    a.add("phib", MPHI, 1)
    for i in range(2):
        a.add(f"gww{i}", 128, C)
    for i in range(4):
        a.add(f"fwT{i}", 128, C)
    return a


def _mk_layout_s():
    a = _PackAlloc()
    for nm, k in (("wq", DE), ("wk", DE), ("wve", KE), ("wo", DE)):
        for i, (o, s) in enumerate(_ksl(k)):
            a.add(f"{nm}{i}", s, DE)
    for i, (o, s) in enumerate(_ksl(DE)):
        a.add(f"gc1{i}", s, C)
    for i in range(2):
        a.add(f"gc2{i}", 128, C)
    for i, (o, s) in enumerate(_ksl(KE)):
        a.add(f"embTe{i}", s, NN)
    a.add("emb", NN, DE, stack=True)
    a.add("bo", 1, DE, stack=True)
    a.add("adj", NN, NN, stack=True)
    for nm, k in (("bq", DE), ("bk", DE)):
        for i, (o, s) in enumerate(_ksl(k)):
            a.add(f"{nm}{i}", s, 1, stack=True)
    return a


_LH = _mk_layout_h()
_LS = _mk_layout_s()


def _pack_h(inputs):
    f = lambda k: np.ascontiguousarray(inputs[k], dtype=np.float32)
    img = np.zeros((128, _LH.ncols), np.float32)

    def put(name, arr):
        r, c, rows, cols = _LH.blocks[name]
        img[r:r + rows, c:c + cols] = arr

    phiwT = f("phi_w").T
    globwT = f("glob_w").T
    for i, (o, s) in enumerate(_ksl(C)):
        put(f"phiwT{i}", phiwT[o:o + s])
        put(f"globwT{i}", globwT[o:o + s])
        put(f"win{i}", f("s2l_w")[:C].reshape(C, 1)[o:o + s])
        put(f"gww{i}", f("gw_w")[o:o + s])
    put("phib", f("phi_b").reshape(MPHI, 1))
    fwT = f("final_w").T
    for i, (o, s) in enumerate(_ksl(2 * C)):
        put(f"fwT{i}", fwT[o:o + s])
    return img


def _pack_s(inputs):
    bf = mybir.dt.np(BF)
    f = lambda k: np.ascontiguousarray(inputs[k], dtype=np.float32)
    img = np.zeros((128, _LS.ncols), bf)

    def put(name, arr):
        r, c, rows, cols = _LS.blocks[name]
        img[r:r + rows, c:c + cols] = arr.astype(bf)

    wve = np.vstack([f("wv"), f("bv")[None, :]])
    embTe = np.vstack([f("emb").T, np.ones((1, NN), np.float32)])
    for nm, k, arr in (("wq", DE, f("wq")), ("wk", DE, f("wk")),
                       ("wve", KE, wve), ("wo", DE, f("wo")),
                       ("gc1", DE, f("gc1_w")), ("embTe", KE, embTe),
                       ("bq", DE, f("bq").reshape(DE, 1)),
                       ("bk", DE, f("bk").reshape(DE, 1))):
        for i, (o, s) in enumerate(_ksl(k)):
            put(f"{nm}{i}", arr[o:o + s])
    for i, (o, s) in enumerate(_ksl(C)):
        put(f"gc2{i}", f("gc2_w")[o:o + s])
    put("emb", f("emb"))
    put("bo", f("bo").reshape(1, DE))
    put("adj", f("adj"))
    return img


# ---------------------------------------------------------------------------
# kernel build
# ---------------------------------------------------------------------------

def _build_nc():
    nc = bacc.Bacc()

    x_p = nc.declare_dram_parameter("x", [BPC, C * HW], FP, isOutput=False)
    out_p = nc.declare_dram_parameter("out", [BPC, C * HW], FP, isOutput=True)
    ph_p = nc.declare_dram_parameter("wpackH", [128, _LH.ncols], FP,
                                     isOutput=False)
    ps_p = nc.declare_dram_parameter("wpackS", [128, _LS.ncols], BF,
                                     isOutput=False)
    rscr = nc.dram_tensor("rscratch", [BPC, MPHI * HW], FP)

    with tile.TileContext(nc) as tc:
        with nc.allow_low_precision(reason="float32r/bf16 matmul feeds"), \
             ExitStack() as ctx:
            _body(ctx, tc, nc, x_p, out_p, ph_p, ps_p, rscr)
    nc.finalize()
    return nc


def _body(ctx, tc, nc, x_p, out_p, ph_p, ps_p, rscr):
    cw = ctx.enter_context(tc.tile_pool(name="cw", bufs=1))      # persistent
    sem = ctx.enter_context(tc.tile_pool(name="sem", bufs=1))    # semantic
    sm = ctx.enter_context(tc.tile_pool(name="sm", bufs=2))      # small/batch
    xm = ctx.enter_context(tc.tile_pool(name="xm", bufs=2))
    xvp = ctx.enter_context(tc.tile_pool(name="xvp", bufs=2))
    rp = ctx.enter_context(tc.tile_pool(name="rp", bufs=2))
    etp = ctx.enter_context(tc.tile_pool(name="etp", bufs=16))
    spp = ctx.enter_context(tc.tile_pool(name="spp", bufs=2))
    obp = ctx.enter_context(tc.tile_pool(name="obp", bufs=2))
    ps_w = ctx.enter_context(tc.tile_pool(name="ps_w", bufs=2, space="PSUM"))
    ps_x = ctx.enter_context(tc.tile_pool(name="ps_x", bufs=2, space="PSUM"))
    ps_t = ctx.enter_context(tc.tile_pool(name="ps_t", bufs=2, space="PSUM"))

    def mm(out, lhsT, rhs, start, stop):
        nc.tensor.matmul(out, _fr(lhsT), _fr(rhs), start=start, stop=stop)

    def mmb(out, lhsT, rhs, start, stop):
        nc.tensor.matmul(out, lhsT, rhs, start=start, stop=stop)

    # ---------------- phase A: constants + input DMAs ----------------
    ident = cw.tile([128, 128], FP, tag="ident")
    masks.make_identity(nc, ident[:])

    packH = cw.tile([128, _LH.ncols], FP, tag="packH")
    nc.sync.dma_start(_fr(packH[:]), _fr(ph_p[:]))

    def wh(name):
        r, c, rows, cols = _LH.blocks[name]
        return packH[r:r + rows, c:c + cols]

    packS = cw.tile([128, _LS.ncols], BF, tag="packS")

    def ws(name):
        r, c, rows, cols = _LS.blocks[name]
        return packS[r:r + rows, c:c + cols]

    # Me tiles are persistent so their constant ones-row (row 16) is
    # written once here.
    Me = [cw.tile([MPHI + 1, HW], FP, tag=f"Me{b}", name=f"Me{b}")
          for b in range(BPC)]
    for b in range(BPC):
        nc.gpsimd.memset(Me[b][MPHI:MPHI + 1, :], 1.0)
    one_row_b = cw.tile([1, NN], BF, tag="one_row_b")
    nc.gpsimd.memset(one_row_b[:], 1.0)
    ones20b = cw.tile([NN, 8], BF, tag="ones20b")
    nc.gpsimd.memset(ones20b[:], 1.0)
    inv20b = cw.tile([NN, 8], BF, tag="inv20b")
    nc.gpsimd.memset(inv20b[:], 1.0 / NN)

    x_mat = x_p[:].rearrange("b (c2 c q) -> b c c2 q", c2=2, c=128, q=HW)
    x_raw = x_p[:].rearrange("b (t q c) -> b q t c", t=8, q=128, c=C)
    out_v = out_p[:].rearrange("b (o2 o q) -> b o o2 q", o2=2, o=128, q=HW)
    r_st = rscr[:].rearrange("b (j q) -> b j q", j=MPHI)
    r_ld = rscr[:].rearrange("b (t p m) -> b p t m", t=8, p=128, m=MPHI)

    xmat, xv, xvv = [], [], []
    for b in range(BPC):
        t = xm.tile([128, 2 * HW], FP, tag="xmat")
        nc.sync.dma_start(_fr(t[:].rearrange("c (c2 q) -> c c2 q", c2=2)),
                          _fr(x_mat[b]))
        xmat.append(t)
    for b in range(BPC):
        t = xvp.tile([128, 8 * (C + 1)], FP, tag="xv")
        v = t[:].rearrange("q (t c) -> q t c", t=8)
        nc.sync.dma_start(_fr(v[:, :, 0:C]), _fr(x_raw[b]))
        nc.gpsimd.memset(v[:, :, C:C + 1], 1.0)
        xv.append(t)
        xvv.append(v)

    nc.sync.dma_start(packS[:], ps_p[:])

    # ---------------- phase B: per-batch preamble ----------------
    R, Dg, negMm = [], [], []
    for b in range(BPC):
        xmv = xmat[b][:].rearrange("c (c2 q) -> c c2 q", c2=2)
        # phi = phi_w @ x + phi_b ; R = relu(phi)
        pphi = ps_w.tile([MPHI, HW], FP, tag="ps_w")
        for ki in range(2):
            for nh in range(2):
                mm(pphi[:, 512 * nh:512 * (nh + 1)], wh(f"phiwT{ki}"),
                   xmv[:, ki, 512 * nh:512 * (nh + 1)],
                   start=(ki == 0), stop=(ki == 1))
        Rb = rp.tile([MPHI, HW], FP, tag="R")
        nc.scalar.activation(_fr(Rb[:]), pphi[:], AF.Relu,
                             bias=wh("phib")[:, 0:1])
        nc.sync.dma_start(r_st[b], Rb[:])
        R.append(Rb)

        # g = glob_w @ mean(x); Dg = sigmoid-diag trick
        xmean = sm.tile([128, 2], FP, tag="xmean")
        for ki in range(2):
            nc.vector.tensor_reduce(xmean[:, ki:ki + 1], xmv[:, ki, :],
                                    axis=AX.X, op=ALU.add)
        pg = ps_t.tile([MPHI, 1], FP, tag="ps_t")
        for ki in range(2):
            mm(pg[:], wh(f"globwT{ki}"), xmean[:, ki:ki + 1],
               start=(ki == 0), stop=(ki == 1))
        sm05 = sm.tile([MPHI, 1], FP, tag="sm05")
        nc.scalar.activation(sm05[:], pg[:, 0:1], AF.Sigmoid,
                             scale=float(1.0 / HW))
        nc.gpsimd.tensor_scalar_add(sm05[:], sm05[:], -0.5)
        Dgb = sm.tile([MPHI, MPHI], FP, tag="Dg")
        nc.gpsimd.tensor_scalar(_fr(Dgb[:]), ident[0:MPHI, 0:MPHI],
                                sm05[:, 0:1], 0.5, op0=ALU.mult, op1=ALU.add)
        Dg.append(Dgb)

        # M rows 0:16 = Dg @ R (row 16 is the persistent ones row)
        pm = ps_w.tile([MPHI, HW], FP, tag="ps_w")
        for nh in range(2):
            mm(pm[:, 512 * nh:512 * (nh + 1)], Dgb[:],
               Rb[:, 512 * nh:512 * (nh + 1)], start=True, stop=True)
        nc.vector.tensor_copy(_fr(Me[b][0:MPHI, :]), pm[:])
        Mmax = sm.tile([MPHI, 1], FP, tag="Mmax")
        nc.vector.tensor_reduce(Mmax[:], pm[:], axis=AX.X, op=ALU.max)
        nMm = sm.tile([MPHI, 1], FP, tag="negMm")
        nc.vector.tensor_scalar_mul(_fr(nMm[:]), Mmax[:], -1.0)
        negMm.append(nMm)

    # ---------------- phase C: x_phi reload + transpose + ub row ----------
    xpT = []
    for b in range(BPC):
        xpa = sm.tile([128, 128], FP, tag="xpa")
        nc.sync.dma_start(
            xpa[:].rearrange("p (t m) -> p t m", t=8), r_ld[b])
        xt = rp.tile([MPHI + 1, HW], FP, tag="xpT")
        for h in range(2):
            psx = ps_t.tile([MPHI, 512], FP, tag="ps_t")
            for j in range(4):
                t8 = 4 * h + j
                nc.tensor.transpose(_fr(psx[:, 128 * j:128 * (j + 1)]),
                                    _fr(xpa[:, MPHI * t8:MPHI * (t8 + 1)]),
                                    _fr(ident[:, :]))
            nc.vector.tensor_copy(_fr(xt[0:MPHI, 512 * h:512 * (h + 1)]),
                                  psx[:])
        pub = ps_w.tile([1, HW], FP, tag="ps_w")
        for nh in range(2):
            mm(pub[:, 512 * nh:512 * (nh + 1)], negMm[b][:, 0:1],
               xt[0:MPHI, 512 * nh:512 * (nh + 1)], start=True, stop=True)
        nc.scalar.copy(_fr(xt[MPHI:MPHI + 1, :]), pub[:])
        xpT.append(xt)

    # ---------------- semantic branch (batch independent, bf16) ----------
    # qT/kT [300, 20] chunks: qT = wq^T @ emb^T + bias col
    def qt_like(wname, bname, tag):
        outs = []
        for mi, (mo, ms) in enumerate(_ksl(DE)):
            ps = ps_t.tile([ms, NN], FP, tag="ps_t")
            for ki, (ko, ks) in enumerate(_ksl(DE)):
                mmb(ps[:], ws(f"{wname}{ki}")[:, mo:mo + ms],
                    ws(f"embTe{ki}")[0:ks, :], start=(ki == 0), stop=(ki == 2))
            t = sem.tile([ms, NN], BF, tag=f"{tag}{mi}")
            nc.scalar.activation(t[:], ps[:], AF.Identity,
                                 bias=ws(f"{bname}{mi}")[:, 0:1])
            outs.append(t)
        return outs

    qT = qt_like("wq", "bq", "qT")
    kT = qt_like("wk", "bk", "kT")

    # v natural [20, 300] = embTe.T @ wve (bias row fused)
    ps = ps_t.tile([NN, DE], FP, tag="ps_t")
    for ki in range(3):
        mmb(ps[:], ws(f"embTe{ki}"), ws(f"wve{ki}"),
            start=(ki == 0), stop=(ki == 2))
    v_sb = sem.tile([NN, DE], BF, tag="v_sb")
    nc.vector.tensor_copy(v_sb[:], ps[:])

    # att = softmax(q @ k.T / sqrt(300)) : [20, 20]
    ps = ps_t.tile([NN, NN], FP, tag="ps_t")
    for ki in range(3):
        mmb(ps[:], qT[ki][:], kT[ki][:], start=(ki == 0), stop=(ki == 2))
    att_s = sem.tile([NN, NN], FP, tag="att_s")
    nc.scalar.activation(att_s[:], ps[:], AF.Identity,
                         scale=float(1.0 / np.sqrt(DE)))
    mx = sem.tile([NN, 1], FP, tag="mx")
    nc.vector.tensor_reduce(mx[:], att_s[:], axis=AX.X, op=ALU.max)
    negmx = sem.tile([NN, 1], FP, tag="negmx")
    nc.vector.tensor_scalar_mul(negmx[:], mx[:], -1.0)
    att_e = sem.tile([NN, NN], FP, tag="att_e")
    rs = sem.tile([NN, 1], FP, tag="rs")
    nc.scalar.activation(att_e[:], att_s[:], AF.Exp, bias=negmx[:, 0:1],
                         accum_out=rs[:, 0:1])
    rr = sem.tile([NN, 1], FP, tag="rr")
    nc.vector.reciprocal(rr[:], rs[:])
    att_n = sem.tile([NN, NN], FP, tag="att_n")
    nc.vector.tensor_scalar_mul(att_n[:], att_e[:], rr[:, 0:1])

    # attT, AV = att @ v, node1 col [300] (as 3 chunks in n1c cols)
    ps = ps_t.tile([NN, NN], FP, tag="ps_t")
    nc.tensor.transpose(ps[:], att_n[:], ident[0:NN, 0:NN])
    attT = sem.tile([NN, NN], BF, tag="attT")
    nc.vector.tensor_copy(attT[:], ps[:])
    ps = ps_t.tile([NN, DE], FP, tag="ps_t")
    mmb(ps[:], attT[:], v_sb[:], start=True, stop=True)
    av_sb = sem.tile([NN, DE], BF, tag="av_sb")
    nc.vector.tensor_copy(av_sb[:], ps[:])

    n1c = sem.tile([128, 3], BF, tag="n1c")
    for mi, (mo, ms) in enumerate(_ksl(DE)):
        ps = ps_t.tile([ms, 8], FP, tag="ps_t")
        mmb(ps[:], av_sb[:, mo:mo + ms], inv20b[:], start=True, stop=True)
        nc.vector.tensor_copy(n1c[0:ms, mi:mi + 1], ps[:, 0:1])

    # node2 [1,300] = node1^T @ wo + bo ; ev = emb + bcast(node2)
    ps = ps_t.tile([1, DE], FP, tag="ps_t")
    for ki, (ko, ks) in enumerate(_ksl(DE)):
        mmb(ps[:], n1c[0:ks, ki:ki + 1], ws(f"wo{ki}"),
            start=(ki == 0), stop=(ki == 2))
    n2 = sem.tile([1, DE], BF, tag="n2")
    nc.vector.tensor_add(n2[:], ws("bo"), ps[:])
    ps = ps_t.tile([NN, DE], FP, tag="ps_t")
    mmb(ps[:], one_row_b[:], n2[:], start=True, stop=True)
    ev_sb = sem.tile([NN, DE], FP, tag="ev_sb")
    nc.vector.tensor_add(ev_sb[:], ws("emb"), ps[:])

    evT = []
    for mi, (mo, ms) in enumerate(_ksl(DE)):
        ps = ps_t.tile([ms, NN], FP, tag="ps_t")
        nc.tensor.transpose(ps[:], ev_sb[:, mo:mo + ms], ident[0:NN, 0:NN])
        t = sem.tile([ms, NN], BF, tag=f"evT{mi}")
        nc.vector.tensor_copy(t[:], ps[:])
        evT.append(t)

    # adj_n = (d (x) d) * (adj + I)
    ah = sem.tile([NN, NN], FP, tag="ah")
    nc.gpsimd.tensor_add(ah[:], ws("adj"), ident[0:NN, 0:NN])
    r20 = sem.tile([NN, 1], FP, tag="r20")
    nc.vector.tensor_reduce(r20[:], ah[:], axis=AX.X, op=ALU.add)
    ir20 = sem.tile([NN, 1], FP, tag="ir20")
    nc.vector.reciprocal(ir20[:], r20[:])
    d20 = sem.tile([NN, 1], FP, tag="d20")
    nc.scalar.activation(d20[:], ir20[:], AF.Sqrt)
    ps = ps_t.tile([1, NN], FP, tag="ps_t")
    nc.tensor.transpose(ps[:], d20[:, 0:1], ident[0:NN, 0:NN])
    dT = sem.tile([1, NN], FP, tag="dT")
    nc.vector.tensor_copy(_fr(dT[:]), ps[:])
    ps = ps_t.tile([NN, NN], FP, tag="ps_t")
    mm(ps[:], dT[:], dT[:], start=True, stop=True)
    adjn = sem.tile([NN, NN], FP, tag="adjn")
    nc.vector.tensor_mul(adjn[:], ah[:], ps[:])
    ps = ps_t.tile([NN, NN], FP, tag="ps_t")
    nc.tensor.transpose(ps[:], adjn[:], ident[0:NN, 0:NN])
    adjnT = sem.tile([NN, NN], BF, tag="adjnT")
    nc.vector.tensor_copy(adjnT[:], ps[:])

    # GCN: g2 = relu(adj_n @ (relu(adj_n @ (ev @ gc1)) @ gc2))
    ps = ps_t.tile([NN, C], FP, tag="ps_t")
    for ki in range(3):
        mmb(ps[:], evT[ki][:], ws(f"gc1{ki}"), start=(ki == 0), stop=(ki == 2))
    t1 = sem.tile([NN, C], BF, tag="t1")
    nc.vector.tensor_copy(t1[:], ps[:])
    ps = ps_t.tile([NN, C], FP, tag="ps_t")
    mmb(ps[:], adjnT[:], t1[:], start=True, stop=True)
    g1 = sem.tile([NN, C], FP, tag="g1")
    nc.vector.tensor_scalar_max(g1[:], ps[:], 0.0)

    g1T = []
    for mi, (mo, ms) in enumerate(_ksl(C)):
        ps = ps_t.tile([ms, NN], FP, tag="ps_t")
        nc.tensor.transpose(ps[:], g1[:, mo:mo + ms], ident[0:NN, 0:NN])
        t = sem.tile([ms, NN], BF, tag=f"g1T{mi}")
        nc.vector.tensor_copy(t[:], ps[:])
        g1T.append(t)

    ps = ps_t.tile([NN, C], FP, tag="ps_t")
    for ki in range(2):
        mmb(ps[:], g1T[ki][:], ws(f"gc2{ki}"), start=(ki == 0), stop=(ki == 1))
    t2 = sem.tile([NN, C], BF, tag="t2")
    nc.vector.tensor_copy(t2[:], ps[:])
    ps = ps_t.tile([NN, C], FP, tag="ps_t")
    mmb(ps[:], adjnT[:], t2[:], start=True, stop=True)
    g2 = sem.tile([NN, C], BF, tag="g2")
    nc.vector.tensor_scalar_max(g2[:], ps[:], 0.0)

    # reluG [128, 2]; fa [1, 256] = reluG^T @ final_w[:, C:].T
    reluG = sem.tile([128, 2], FP, tag="reluG")
    for cb in range(2):
        ps = ps_t.tile([128, 8], FP, tag="ps_t")
        mmb(ps[:], g2[:, 128 * cb:128 * (cb + 1)], ones20b[:],
            start=True, stop=True)
        nc.scalar.activation(_fr(reluG[:, cb:cb + 1]), ps[:, 0:1], AF.Relu)
    ps = ps_t.tile([1, C], FP, tag="ps_t")
    for cb in range(2):
        mm(ps[:], reluG[:, cb:cb + 1], wh(f"fwT{2 + cb}"),
           start=(cb == 0), stop=(cb == 1))
    fa = sem.tile([1, C], FP, tag="fa")
    nc.vector.tensor_copy(_fr(fa[:]), ps[:])

    # WlgT [256, 256]: WlgT[c, o] = sum_k gw_w[k, c] final_w[o, k]
    WlgT = []
    for cb in range(2):
        ps = ps_t.tile([128, C], FP, tag="ps_t")
        for ki in range(2):
            mm(ps[:], wh(f"gww{ki}")[:, 128 * cb:128 * (cb + 1)],
               wh(f"fwT{ki}"), start=(ki == 0), stop=(ki == 1))
        t = sem.tile([128, C], FP, tag=f"WlgT{cb}")
        nc.vector.tensor_copy(_fr(t[:]), ps[:])
        WlgT.append(t)

    # ---------------- phase D: sa softmax numerator + scaled fa ----------
    ea, fab = [], []
    for b in range(BPC):
        xmv = xmat[b][:].rearrange("c (c2 q) -> c c2 q", c2=2)
        pa = ps_w.tile([1, HW], FP, tag="ps_w")
        for ki in range(2):
            for nh in range(2):
                mm(pa[:, 512 * nh:512 * (nh + 1)], wh(f"win{ki}"),
                   xmv[:, ki, 512 * nh:512 * (nh + 1)],
                   start=(ki == 0), stop=(ki == 1))
        eab = rp.tile([1, HW], FP, tag="ea")
        sae = sm.tile([1, 1], FP, tag="sae")
        nc.scalar.activation(eab[:], pa[:], AF.Exp, accum_out=sae[:, 0:1])
        sar = sm.tile([1, 1], FP, tag="sar")
        nc.vector.reciprocal(sar[:], sae[:])
        fb = sm.tile([1, C], FP, tag="fab")
        nc.vector.tensor_scalar_mul(_fr(fb[:]), fa[:], sar[0:1, 0:1])
        ea.append(eab)
        fab.append(fb)

    # ---------------- phases E/F/G per batch: ET, EXV+spiral^T, out ------
    for b in range(BPC):
        # E^T tiles [128, 1024] = exp(S^T - ub)
        ET = []
        for t8 in range(8):
            pst = ps_w.tile([128, HW], FP, tag="ps_w")
            for nh in range(2):
                mm(pst[:, 512 * nh:512 * (nh + 1)],
                   Me[b][:, 128 * t8:128 * (t8 + 1)],
                   xpT[b][:, 512 * nh:512 * (nh + 1)], start=True, stop=True)
            et = etp.tile([128, HW], FP, tag="et")
            nc.scalar.activation(_fr(et[:]), pst[:], AF.Exp)
            ET.append(et)

        # per p-tile: EXV (col 256 = D); spiral; transpose into spT
        spT = spp.tile([128, 2 * HW], FP, tag="spT")
        spTv = spT[:].rearrange("c (ch p) -> c ch p", ch=2)
        for pt in range(8):
            pe = ps_x.tile([128, C + 1], FP, tag="ps_x")
            for k in range(8):
                mm(pe[:], ET[k][:, 128 * pt:128 * (pt + 1)], xv[b][:, 257 * k:257 * k + 257],
                   start=(k == 0), stop=(k == 7))
            negD = sm.tile([128, 1], FP, tag="negD")
            nc.vector.tensor_scalar_mul(negD[:], pe[:, C:C + 1], -1.0)
            nrd = sm.tile([128, 1], FP, tag="nrd")
            nc.vector.reciprocal(nrd[:], negD[:])
            spr = sm.tile([128, C], FP, tag="spr")
            nc.vector.scalar_tensor_tensor(spr[:], pe[:, 0:C], nrd[:, 0:1],
                                           xvv[b][:, pt, 0:C],
                                           op0=ALU.mult, op1=ALU.add)
            ptr = ps_x.tile([128, C], FP, tag="ps_x")
            for ch in range(2):
                nc.tensor.transpose(_fr(ptr[:, 128 * ch:128 * (ch + 1)]),
                                    _fr(spr[:, 128 * ch:128 * (ch + 1)]),
                                    _fr(ident[:, :]))
            nc.vector.tensor_copy(
                _fr(spTv[:, :, 128 * pt:128 * (pt + 1)]),
                ptr[:].rearrange("p (ch q) -> p ch q", ch=2))

        # out[o,:] = relu(Wlg @ spiral^T + fa (x) ea + x)
        xmv = xmat[b][:].rearrange("c (c2 q) -> c c2 q", c2=2)
        ob = obp.tile([128, 2 * HW], FP, tag="ob")
        for ot in range(2):
            po = ps_w.tile([128, HW], FP, tag="ps_w")
            for nh in range(2):
                sl = slice(512 * nh, 512 * (nh + 1))
                for ct in range(2):
                    mm(po[:, sl], WlgT[ct][:, 128 * ot:128 * (ot + 1)],
                       spTv[:, ct, sl], start=(ct == 0), stop=False)
                mm(po[:, sl], fab[b][0:1, 128 * ot:128 * (ot + 1)],
                   ea[b][0:1, sl], start=False, stop=False)
                mm(po[:, sl], ident[:, :], xmv[:, ot, sl],
                   start=False, stop=True)
            nc.scalar.activation(_fr(ob[:, HW * ot:HW * (ot + 1)]), po[:],
                                 AF.Relu)
        nc.sync.dma_start(
            _fr(out_v[b]),
            _fr(ob[:].rearrange("o (o2 q) -> o o2 q", o2=2)))


# ---------------------------------------------------------------------------
# host driver
# ---------------------------------------------------------------------------

def _prep_shared(inputs):
    return {"wpackH": _pack_h(inputs), "wpackS": _pack_s(inputs)}


_NC_CACHE = {}


def kernel(**inputs):
    global LAST_EXEC_NS, LAST_RESULT
    if "nc" not in _NC_CACHE:
        _NC_CACHE["nc"] = _build_nc()
    nc = _NC_CACHE["nc"]

    x = np.ascontiguousarray(inputs["x"], dtype=np.float32)
    B = x.shape[0]
    shared = _prep_shared(inputs)
    in_maps = []
    for i in range(NCORES):
        m = dict(shared)
        m["x"] = np.ascontiguousarray(
            x[i * BPC:(i + 1) * BPC].reshape(BPC, C * HW))
        in_maps.append(m)

    trace = os.environ.get("KERNEL_TRACE", "0") == "1"
    res = run_bass_kernel_spmd(nc, in_maps, list(range(NCORES)), trace=trace)
    LAST_RESULT = res
    LAST_EXEC_NS = getattr(res, "exec_time_ns", None)

    out = np.empty((B, C, 32, 32), np.float32)
    for i in range(NCORES):
        out[i * BPC:(i + 1) * BPC] = res.results[i]["out"].reshape(BPC, C, 32, 32)
    return out


# revision 10
# speedup vs baseline: 1.5578x; 1.0938x over previous
"""Trainium2 Bass kernel for the CDGR gnn_message_passing module.

Mathematically exact reformulation of the reference (see derivation in the
docstrings below):

  - softmax rows of A sum to 1  =>  L = I - A, the d-scaling vanishes
  - s2l logits are additively separable in (pixel, node) => the softmax
    over pixels is identical for every node column => app collapses to a
    rank-1 outer product relu(G) (x) softmax(w_in . x)
  - the semantic branch (word attention + 2-layer GCN) is batch
    independent => computed once per core (in bf16; it only feeds the
    rank-1 app term and is well inside the 2e-2 tolerance)
  - the two chained 1x1 convs fuse: Wlg = final_w[:, :C] @ gw_w
  - the `+ x` residual is folded into the final matmul as an
    identity-weight accumulation (frees the vector engine)

Per batch (2 per core, data-parallel over 8 cores):
  out[o,q] = relu( Wlg @ spiral^T + fa (x) ea + x )  with
  spiral = xv - (E @ xv) / D,  E = exp(S - ub),  S = x_phi @ Dg @ x_phi_T
  computed via S^T tiles (lhsT = M_ext columns) so that E^T column
  slices feed the big E @ xv matmul directly as lhsT, with a fused ones
  column in xv giving D, and a fused K=17 row giving the -ub shift.

I/O strategy (the previous version spent 114us of SP-sequencer time on 96
small DMAs): all weights/constants are host-packed into two [128, N] DRAM
images (one fp32 "hot" pack, one bf16 "semantic" pack) loaded with one DMA
each, and each batch moves exactly 5 wide strided DMAs (x natural view,
x raw-reshape view, R spill, x_phi reload, output).
"""

import os
from contextlib import ExitStack

import numpy as np

import concourse.bass as bass
import concourse.bacc as bacc
import concourse.mybir as mybir
import concourse.tile as tile
from concourse import masks
from concourse.bass_utils import run_bass_kernel_spmd

FP = mybir.dt.float32
BF = mybir.dt.bfloat16
FR = mybir.dt.float32r
AF = mybir.ActivationFunctionType
ALU = mybir.AluOpType
AX = mybir.AxisListType

NCORES = 8
BPC = 2          # batches per core
C, HW = 256, 1024
MPHI, NN, DE = 16, 20, 300
KE = DE + 1      # 301 = DEMB + fused-bias row

LAST_EXEC_NS = None
LAST_RESULT = None


def _ksl(total, step=128):
    return [(o, min(step, total - o)) for o in range(0, total, step)]


def _fr(ap):
    return ap.bitcast(FR)


# ---------------------------------------------------------------------------
# weight-pack layouts (shared between host packing and kernel build)
# ---------------------------------------------------------------------------

class _PackAlloc:
    """First-fit strip allocator: blocks of equal width stack vertically in a
    128-row strip before opening a new column range."""

    def __init__(self):
        self.strips = []            # [col_off, width, used_rows]
        self.ncols = 0
        self.blocks = {}            # name -> (row, col, rows, cols)

    def add(self, name, rows, cols, stack=False):
        # PE matmul operands must sit at base partition 0 (they pair with
        # base-0 tiles); only non-matmul blocks may stack below other blocks.
        if stack:
            for s in self.strips:
                r = (s[2] + 31) // 32 * 32
                if s[1] == cols and r <= 64 and r + rows <= 128:
                    s[2] = r + rows
                    self.blocks[name] = (r, s[0], rows, cols)
                    return
        off = self.ncols
        self.ncols += cols
        self.strips.append([off, cols, rows])
        self.blocks[name] = (0, off, rows, cols)


def _mk_layout_h():
    a = _PackAlloc()
    for i in range(2):
        a.add(f"phiwT{i}", 128, MPHI)
    for i in range(2):
        a.add(f"globwT{i}", 128, MPHI)
    for i in range(2):
        a.add(f"win{i}", 128, 1)
    a.add("phib", MPHI, 1)
    for i in range(2):
        a.add(f"gww{i}", 128, C)
    for i in range(4):
        a.add(f"fwT{i}", 128, C)
    return a


def _mk_layout_s():
    a = _PackAlloc()
    for nm, k in (("wq", DE), ("wk", DE), ("wve", KE), ("wo", DE)):
        for i, (o, s) in enumerate(_ksl(k)):
            a.add(f"{nm}{i}", s, DE)
    for i, (o, s) in enumerate(_ksl(DE)):
        a.add(f"gc1{i}", s, C)
    for i in range(2):
        a.add(f"gc2{i}", 128, C)
    for i, (o, s) in enumerate(_ksl(KE)):
        a.add(f"embTe{i}", s, NN)
    a.add("emb", NN, DE, stack=True)
    a.add("bo", 1, DE, stack=True)
    a.add("adj", NN, NN, stack=True)
    for nm, k in (("bq", DE), ("bk", DE)):
        for i, (o, s) in enumerate(_ksl(k)):
            a.add(f"{nm}{i}", s, 1, stack=True)
    return a


_LH = _mk_layout_h()
_LS = _mk_layout_s()


def _pack_h(inputs):
    f = lambda k: np.ascontiguousarray(inputs[k], dtype=np.float32)
    img = np.zeros((128, _LH.ncols), np.float32)

    def put(name, arr):
        r, c, rows, cols = _LH.blocks[name]
        img[r:r + rows, c:c + cols] = arr

    phiwT = f("phi_w").T
    globwT = f("glob_w").T
    for i, (o, s) in enumerate(_ksl(C)):
        put(f"phiwT{i}", phiwT[o:o + s])
        put(f"globwT{i}", globwT[o:o + s])
        put(f"win{i}", f("s2l_w")[:C].reshape(C, 1)[o:o + s])
        put(f"gww{i}", f("gw_w")[o:o + s])
    put("phib", f("phi_b").reshape(MPHI, 1))
    fwT = f("final_w").T
    for i, (o, s) in enumerate(_ksl(2 * C)):
        put(f"fwT{i}", fwT[o:o + s])
    return img


def _pack_s(inputs):
    bf = mybir.dt.np(BF)
    f = lambda k: np.ascontiguousarray(inputs[k], dtype=np.float32)
    img = np.zeros((128, _LS.ncols), bf)

    def put(name, arr):
        r, c, rows, cols = _LS.blocks[name]
        img[r:r + rows, c:c + cols] = arr.astype(bf)

    wve = np.vstack([f("wv"), f("bv")[None, :]])
    embTe = np.vstack([f("emb").T, np.ones((1, NN), np.float32)])
    for nm, k, arr in (("wq", DE, f("wq")), ("wk", DE, f("wk")),
                       ("wve", KE, wve), ("wo", DE, f("wo")),
                       ("gc1", DE, f("gc1_w")), ("embTe", KE, embTe),
                       ("bq", DE, f("bq").reshape(DE, 1)),
                       ("bk", DE, f("bk").reshape(DE, 1))):
        for i, (o, s) in enumerate(_ksl(k)):
            put(f"{nm}{i}", arr[o:o + s])
    for i, (o, s) in enumerate(_ksl(C)):
        put(f"gc2{i}", f("gc2_w")[o:o + s])
    put("emb", f("emb"))
    put("bo", f("bo").reshape(1, DE))
    put("adj", f("adj"))
    return img


# ---------------------------------------------------------------------------
# kernel build
# ---------------------------------------------------------------------------

def _build_nc():
    nc = bacc.Bacc()

    x_p = nc.declare_dram_parameter("x", [BPC, C * HW], FP, isOutput=False)
    out_p = nc.declare_dram_parameter("out", [BPC, C * HW], FP, isOutput=True)
    ph_p = nc.declare_dram_parameter("wpackH", [128, _LH.ncols], FP,
                                     isOutput=False)
    ps_p = nc.declare_dram_parameter("wpackS", [128, _LS.ncols], BF,
                                     isOutput=False)
    rscr = nc.dram_tensor("rscratch", [BPC, MPHI * HW], FP)

    with tile.TileContext(nc) as tc:
        with nc.allow_low_precision(reason="float32r/bf16 matmul feeds"), \
             ExitStack() as ctx:
            _body(ctx, tc, nc, x_p, out_p, ph_p, ps_p, rscr)
    nc.finalize()
    return nc


def _body(ctx, tc, nc, x_p, out_p, ph_p, ps_p, rscr):
    cw = ctx.enter_context(tc.tile_pool(name="cw", bufs=1))      # persistent
    sem = ctx.enter_context(tc.tile_pool(name="sem", bufs=1))    # semantic
    sm = ctx.enter_context(tc.tile_pool(name="sm", bufs=2))      # small/batch
    xm = ctx.enter_context(tc.tile_pool(name="xm", bufs=2))
    xvp = ctx.enter_context(tc.tile_pool(name="xvp", bufs=2))
    rp = ctx.enter_context(tc.tile_pool(name="rp", bufs=2))
    etp = ctx.enter_context(tc.tile_pool(name="etp", bufs=16))
    spp = ctx.enter_context(tc.tile_pool(name="spp", bufs=2))
    obp = ctx.enter_context(tc.tile_pool(name="obp", bufs=4))
    ps_w = ctx.enter_context(tc.tile_pool(name="ps_w", bufs=2, space="PSUM"))
    ps_x = ctx.enter_context(tc.tile_pool(name="ps_x", bufs=2, space="PSUM"))
    ps_t = ctx.enter_context(tc.tile_pool(name="ps_t", bufs=2, space="PSUM"))

    def mm(out, lhsT, rhs, start, stop):
        nc.tensor.matmul(out, _fr(lhsT), _fr(rhs), start=start, stop=stop)

    def mmb(out, lhsT, rhs, start, stop):
        nc.tensor.matmul(out, lhsT, rhs, start=start, stop=stop)

    # ---------------- phase A: constants + input DMAs ----------------
    ident = cw.tile([128, 128], FP, tag="ident")
    masks.make_identity(nc, ident[:])

    packH = cw.tile([128, _LH.ncols], FP, tag="packH")
    nc.sync.dma_start(_fr(packH[:]), _fr(ph_p[:]))

    def wh(name):
        r, c, rows, cols = _LH.blocks[name]
        return packH[r:r + rows, c:c + cols]

    packS = cw.tile([128, _LS.ncols], BF, tag="packS")

    def ws(name):
        r, c, rows, cols = _LS.blocks[name]
        return packS[r:r + rows, c:c + cols]

    # Me tiles are persistent so their constant ones-row (row 16) is
    # written once here.
    Me = [cw.tile([MPHI + 1, HW], FP, tag=f"Me{b}", name=f"Me{b}")
          for b in range(BPC)]
    for b in range(BPC):
        nc.gpsimd.memset(Me[b][MPHI:MPHI + 1, :], 1.0)
    one_row_b = cw.tile([1, NN], BF, tag="one_row_b")
    nc.gpsimd.memset(one_row_b[:], 1.0)
    ones20b = cw.tile([NN, 8], BF, tag="ones20b")
    nc.gpsimd.memset(ones20b[:], 1.0)
    inv20b = cw.tile([NN, 8], BF, tag="inv20b")
    nc.gpsimd.memset(inv20b[:], 1.0 / NN)

    x_mat = x_p[:].rearrange("b (c2 c q) -> b c c2 q", c2=2, c=128, q=HW)
    x_raw = x_p[:].rearrange("b (t q c) -> b q t c", t=8, q=128, c=C)
    out_v = out_p[:].rearrange("b (o2 o q) -> b o o2 q", o2=2, o=128, q=HW)
    r_st = rscr[:].rearrange("b (j q) -> b j q", j=MPHI)
    r_ld = rscr[:].rearrange("b (t p m) -> b p t m", t=8, p=128, m=MPHI)

    xmat, xv, xvv = [], [], []
    for b in range(BPC):
        t = xm.tile([128, 2 * HW], FP, tag="xmat")
        nc.sync.dma_start(_fr(t[:].rearrange("c (c2 q) -> c c2 q", c2=2)),
                          _fr(x_mat[b]))
        xmat.append(t)
    for b in range(BPC):
        t = xvp.tile([128, 8 * (C + 1)], FP, tag="xv")
        v = t[:].rearrange("q (t c) -> q t c", t=8)
        nc.sync.dma_start(_fr(v[:, :, 0:C]), _fr(x_raw[b]))
        nc.gpsimd.memset(v[:, :, C:C + 1], 1.0)
        xv.append(t)
        xvv.append(v)

    nc.sync.dma_start(packS[:], ps_p[:])

    # ---------------- phase B: per-batch preamble ----------------
    R, Dg, negMm = [], [], []
    for b in range(BPC):
        xmv = xmat[b][:].rearrange("c (c2 q) -> c c2 q", c2=2)
        # phi = phi_w @ x + phi_b ; R = relu(phi)
        pphi = ps_w.tile([MPHI, HW], FP, tag="ps_w")
        for ki in range(2):
            for nh in range(2):
                mm(pphi[:, 512 * nh:512 * (nh + 1)], wh(f"phiwT{ki}"),
                   xmv[:, ki, 512 * nh:512 * (nh + 1)],
                   start=(ki == 0), stop=(ki == 1))
        Rb = rp.tile([MPHI, HW], FP, tag="R")
        nc.scalar.activation(_fr(Rb[:]), pphi[:], AF.Relu,
                             bias=wh("phib")[:, 0:1])
        nc.gpsimd.dma_start(r_st[b], Rb[:])
        R.append(Rb)

        # g = glob_w @ mean(x); Dg = sigmoid-diag trick
        xmean = sm.tile([128, 2], FP, tag="xmean")
        for ki in range(2):
            nc.vector.tensor_reduce(xmean[:, ki:ki + 1], xmv[:, ki, :],
                                    axis=AX.X, op=ALU.add)
        pg = ps_t.tile([MPHI, 1], FP, tag="ps_t")
        for ki in range(2):
            mm(pg[:], wh(f"globwT{ki}"), xmean[:, ki:ki + 1],
               start=(ki == 0), stop=(ki == 1))
        eng = sm.tile([MPHI, 1], FP, tag="eng")
        nc.scalar.activation(eng[:], pg[:, 0:1], AF.Exp,
                             scale=float(-1.0 / HW))
        nc.gpsimd.tensor_scalar_add(eng[:], eng[:], 1.0)
        sm05 = sm.tile([MPHI, 1], FP, tag="sm05")
        nc.vector.reciprocal(sm05[:], eng[:])
        nc.gpsimd.tensor_scalar_add(sm05[:], sm05[:], -0.5)
        Dgb = sm.tile([MPHI, MPHI], FP, tag="Dg")
        nc.gpsimd.tensor_scalar(_fr(Dgb[:]), ident[0:MPHI, 0:MPHI],
                                sm05[:, 0:1], 0.5, op0=ALU.mult, op1=ALU.add)
        Dg.append(Dgb)

        # M rows 0:16 = Dg @ R (row 16 is the persistent ones row)
        pm = ps_w.tile([MPHI, HW], FP, tag="ps_w")
        for nh in range(2):
            mm(pm[:, 512 * nh:512 * (nh + 1)], Dgb[:],
               Rb[:, 512 * nh:512 * (nh + 1)], start=True, stop=True)
        nc.vector.tensor_copy(_fr(Me[b][0:MPHI, :]), pm[:])
        Mmax = sm.tile([MPHI, 1], FP, tag="Mmax")
        nc.vector.tensor_reduce(Mmax[:], pm[:], axis=AX.X, op=ALU.max)
        nMm = sm.tile([MPHI, 1], FP, tag="negMm")
        nc.vector.tensor_scalar_mul(_fr(nMm[:]), Mmax[:], -1.0)
        negMm.append(nMm)

    # ---------------- phase C: x_phi reload + transpose + ub row ----------
    xpT = []
    for b in range(BPC):
        xpa = sm.tile([128, 128], FP, tag="xpa")
        nc.gpsimd.dma_start(
            xpa[:].rearrange("p (t m) -> p t m", t=8), r_ld[b])
        xt = rp.tile([MPHI + 1, HW], FP, tag="xpT")
        for h in range(2):
            psx = ps_t.tile([MPHI, 512], FP, tag="ps_t")
            for j in range(4):
                t8 = 4 * h + j
                nc.tensor.transpose(_fr(psx[:, 128 * j:128 * (j + 1)]),
                                    _fr(xpa[:, MPHI * t8:MPHI * (t8 + 1)]),
                                    _fr(ident[:, :]))
            nc.vector.tensor_copy(_fr(xt[0:MPHI, 512 * h:512 * (h + 1)]),
                                  psx[:])
        pub = ps_w.tile([1, HW], FP, tag="ps_w")
        for nh in range(2):
            mm(pub[:, 512 * nh:512 * (nh + 1)], negMm[b][:, 0:1],
               xt[0:MPHI, 512 * nh:512 * (nh + 1)], start=True, stop=True)
        nc.scalar.copy(_fr(xt[MPHI:MPHI + 1, :]), pub[:])
        xpT.append(xt)

    # ---------------- semantic branch (batch independent, bf16) ----------
    # qT/kT [300, 20] chunks: qT = wq^T @ emb^T + bias col
    def qt_like(wname, bname, tag):
        outs = []
        for mi, (mo, ms) in enumerate(_ksl(DE)):
            ps = ps_t.tile([ms, NN], FP, tag="ps_t")
            for ki, (ko, ks) in enumerate(_ksl(DE)):
                mmb(ps[:], ws(f"{wname}{ki}")[:, mo:mo + ms],
                    ws(f"embTe{ki}")[0:ks, :], start=(ki == 0), stop=(ki == 2))
            t = sem.tile([ms, NN], BF, tag=f"{tag}{mi}")
            nc.scalar.activation(t[:], ps[:], AF.Identity,
                                 bias=ws(f"{bname}{mi}")[:, 0:1])
            outs.append(t)
        return outs

    qT = qt_like("wq", "bq", "qT")
    kT = qt_like("wk", "bk", "kT")

    # v natural [20, 300] = embTe.T @ wve (bias row fused)
    ps = ps_t.tile([NN, DE], FP, tag="ps_t")
    for ki in range(3):
        mmb(ps[:], ws(f"embTe{ki}"), ws(f"wve{ki}"),
            start=(ki == 0), stop=(ki == 2))
    v_sb = sem.tile([NN, DE], BF, tag="v_sb")
    nc.vector.tensor_copy(v_sb[:], ps[:])

    # att = softmax(q @ k.T / sqrt(300)) : [20, 20]
    ps = ps_t.tile([NN, NN], FP, tag="ps_t")
    for ki in range(3):
        mmb(ps[:], qT[ki][:], kT[ki][:], start=(ki == 0), stop=(ki == 2))
    att_s = sem.tile([NN, NN], FP, tag="att_s")
    nc.scalar.activation(att_s[:], ps[:], AF.Identity,
                         scale=float(1.0 / np.sqrt(DE)))
    mx = sem.tile([NN, 1], FP, tag="mx")
    nc.vector.tensor_reduce(mx[:], att_s[:], axis=AX.X, op=ALU.max)
    negmx = sem.tile([NN, 1], FP, tag="negmx")
    nc.vector.tensor_scalar_mul(negmx[:], mx[:], -1.0)
    att_e = sem.tile([NN, NN], FP, tag="att_e")
    rs = sem.tile([NN, 1], FP, tag="rs")
    nc.scalar.activation(att_e[:], att_s[:], AF.Exp, bias=negmx[:, 0:1],
                         accum_out=rs[:, 0:1])
    rr = sem.tile([NN, 1], FP, tag="rr")
    nc.vector.reciprocal(rr[:], rs[:])
    att_n = sem.tile([NN, NN], FP, tag="att_n")
    nc.vector.tensor_scalar_mul(att_n[:], att_e[:], rr[:, 0:1])

    # attT, AV = att @ v, node1 col [300] (as 3 chunks in n1c cols)
    ps = ps_t.tile([NN, NN], FP, tag="ps_t")
    nc.tensor.transpose(ps[:], att_n[:], ident[0:NN, 0:NN])
    attT = sem.tile([NN, NN], BF, tag="attT")
    nc.vector.tensor_copy(attT[:], ps[:])
    ps = ps_t.tile([NN, DE], FP, tag="ps_t")
    mmb(ps[:], attT[:], v_sb[:], start=True, stop=True)
    av_sb = sem.tile([NN, DE], BF, tag="av_sb")
    nc.vector.tensor_copy(av_sb[:], ps[:])

    n1c = sem.tile([128, 3], BF, tag="n1c")
    for mi, (mo, ms) in enumerate(_ksl(DE)):
        ps = ps_t.tile([ms, 8], FP, tag="ps_t")
        mmb(ps[:], av_sb[:, mo:mo + ms], inv20b[:], start=True, stop=True)
        nc.vector.tensor_copy(n1c[0:ms, mi:mi + 1], ps[:, 0:1])

    # node2 [1,300] = node1^T @ wo + bo ; ev = emb + bcast(node2)
    ps = ps_t.tile([1, DE], FP, tag="ps_t")
    for ki, (ko, ks) in enumerate(_ksl(DE)):
        mmb(ps[:], n1c[0:ks, ki:ki + 1], ws(f"wo{ki}"),
            start=(ki == 0), stop=(ki == 2))
    n2 = sem.tile([1, DE], BF, tag="n2")
    nc.vector.tensor_add(n2[:], ws("bo"), ps[:])
    ps = ps_t.tile([NN, DE], FP, tag="ps_t")
    mmb(ps[:], one_row_b[:], n2[:], start=True, stop=True)
    ev_sb = sem.tile([NN, DE], FP, tag="ev_sb")
    nc.vector.tensor_add(ev_sb[:], ws("emb"), ps[:])

    evT = []
    for mi, (mo, ms) in enumerate(_ksl(DE)):
        ps = ps_t.tile([ms, NN], FP, tag="ps_t")
        nc.tensor.transpose(ps[:], ev_sb[:, mo:mo + ms], ident[0:NN, 0:NN])
        t = sem.tile([ms, NN], BF, tag=f"evT{mi}")
        nc.vector.tensor_copy(t[:], ps[:])
        evT.append(t)

    # adj_n = (d (x) d) * (adj + I)
    ah = sem.tile([NN, NN], FP, tag="ah")
    nc.gpsimd.tensor_add(ah[:], ws("adj"), ident[0:NN, 0:NN])
    r20 = sem.tile([NN, 1], FP, tag="r20")
    nc.vector.tensor_reduce(r20[:], ah[:], axis=AX.X, op=ALU.add)
    ir20 = sem.tile([NN, 1], FP, tag="ir20")
    nc.vector.reciprocal(ir20[:], r20[:])
    d20 = sem.tile([NN, 1], FP, tag="d20")
    nc.scalar.activation(d20[:], ir20[:], AF.Sqrt)
    ps = ps_t.tile([1, NN], FP, tag="ps_t")
    nc.tensor.transpose(ps[:], d20[:, 0:1], ident[0:NN, 0:NN])
    dT = sem.tile([1, NN], FP, tag="dT")
    nc.vector.tensor_copy(_fr(dT[:]), ps[:])
    ps = ps_t.tile([NN, NN], FP, tag="ps_t")
    mm(ps[:], dT[:], dT[:], start=True, stop=True)
    adjn = sem.tile([NN, NN], FP, tag="adjn")
    nc.vector.tensor_mul(adjn[:], ah[:], ps[:])
    ps = ps_t.tile([NN, NN], FP, tag="ps_t")
    nc.tensor.transpose(ps[:], adjn[:], ident[0:NN, 0:NN])
    adjnT = sem.tile([NN, NN], BF, tag="adjnT")
    nc.vector.tensor_copy(adjnT[:], ps[:])

    # GCN: g2 = relu(adj_n @ (relu(adj_n @ (ev @ gc1)) @ gc2))
    ps = ps_t.tile([NN, C], FP, tag="ps_t")
    for ki in range(3):
        mmb(ps[:], evT[ki][:], ws(f"gc1{ki}"), start=(ki == 0), stop=(ki == 2))
    t1 = sem.tile([NN, C], BF, tag="t1")
    nc.vector.tensor_copy(t1[:], ps[:])
    ps = ps_t.tile([NN, C], FP, tag="ps_t")
    mmb(ps[:], adjnT[:], t1[:], start=True, stop=True)
    g1 = sem.tile([NN, C], FP, tag="g1")
    nc.vector.tensor_scalar_max(g1[:], ps[:], 0.0)

    g1T = []
    for mi, (mo, ms) in enumerate(_ksl(C)):
        ps = ps_t.tile([ms, NN], FP, tag="ps_t")
        nc.tensor.transpose(ps[:], g1[:, mo:mo + ms], ident[0:NN, 0:NN])
        t = sem.tile([ms, NN], BF, tag=f"g1T{mi}")
        nc.vector.tensor_copy(t[:], ps[:])
        g1T.append(t)

    ps = ps_t.tile([NN, C], FP, tag="ps_t")
    for ki in range(2):
        mmb(ps[:], g1T[ki][:], ws(f"gc2{ki}"), start=(ki == 0), stop=(ki == 1))
    t2 = sem.tile([NN, C], BF, tag="t2")
    nc.vector.tensor_copy(t2[:], ps[:])
    ps = ps_t.tile([NN, C], FP, tag="ps_t")
    mmb(ps[:], adjnT[:], t2[:], start=True, stop=True)
    g2 = sem.tile([NN, C], BF, tag="g2")
    nc.vector.tensor_scalar_max(g2[:], ps[:], 0.0)

    # reluG [128, 2]; fa [1, 256] = reluG^T @ final_w[:, C:].T
    reluG = sem.tile([128, 2], FP, tag="reluG")
    for cb in range(2):
        ps = ps_t.tile([128, 8], FP, tag="ps_t")
        mmb(ps[:], g2[:, 128 * cb:128 * (cb + 1)], ones20b[:],
            start=True, stop=True)
        nc.scalar.activation(_fr(reluG[:, cb:cb + 1]), ps[:, 0:1], AF.Relu)
    ps = ps_t.tile([1, C], FP, tag="ps_t")
    for cb in range(2):
        mm(ps[:], reluG[:, cb:cb + 1], wh(f"fwT{2 + cb}"),
           start=(cb == 0), stop=(cb == 1))
    fa = sem.tile([1, C], FP, tag="fa")
    nc.vector.tensor_copy(_fr(fa[:]), ps[:])

    # WlgT [256, 256]: WlgT[c, o] = sum_k gw_w[k, c] final_w[o, k]
    WlgT = []
    for cb in range(2):
        ps = ps_t.tile([128, C], FP, tag="ps_t")
        for ki in range(2):
            mm(ps[:], wh(f"gww{ki}")[:, 128 * cb:128 * (cb + 1)],
               wh(f"fwT{ki}"), start=(ki == 0), stop=(ki == 1))
        t = sem.tile([128, C], FP, tag=f"WlgT{cb}")
        nc.vector.tensor_copy(_fr(t[:]), ps[:])
        WlgT.append(t)

    # ---------------- phase D: sa softmax numerator + scaled fa ----------
    ea, fab = [], []
    for b in range(BPC):
        xmv = xmat[b][:].rearrange("c (c2 q) -> c c2 q", c2=2)
        pa = ps_w.tile([1, HW], FP, tag="ps_w")
        for ki in range(2):
            for nh in range(2):
                mm(pa[:, 512 * nh:512 * (nh + 1)], wh(f"win{ki}"),
                   xmv[:, ki, 512 * nh:512 * (nh + 1)],
                   start=(ki == 0), stop=(ki == 1))
        eab = rp.tile([1, HW], FP, tag="ea")
        sae = sm.tile([1, 1], FP, tag="sae")
        nc.scalar.activation(eab[:], pa[:], AF.Exp, accum_out=sae[:, 0:1])
        sar = sm.tile([1, 1], FP, tag="sar")
        nc.vector.reciprocal(sar[:], sae[:])
        fb = sm.tile([1, C], FP, tag="fab")
        nc.vector.tensor_scalar_mul(_fr(fb[:]), fa[:], sar[0:1, 0:1])
        ea.append(eab)
        fab.append(fb)

    # ---------------- phases E/F/G: ET (both batches), then EXV, out ------
    ETb = []
    for b in range(BPC):
        ET = []
        for t8 in range(8):
            pst = ps_w.tile([128, HW], FP, tag="ps_w")
            for nh in range(2):
                mm(pst[:, 512 * nh:512 * (nh + 1)],
                   Me[b][:, 128 * t8:128 * (t8 + 1)],
                   xpT[b][:, 512 * nh:512 * (nh + 1)], start=True, stop=True)
            et = etp.tile([128, HW], FP, tag="et")
            nc.scalar.activation(_fr(et[:]), pst[:], AF.Exp)
            ET.append(et)
        ETb.append(ET)

    for b in range(BPC):
        ET = ETb[b]
        # per p-tile: EXV (col 256 = D); spiral; transpose into spT
        spT = spp.tile([128, 2 * HW], FP, tag="spT")
        spTv = spT[:].rearrange("c (ch p) -> c ch p", ch=2)
        for pt in range(8):
            pe = ps_x.tile([128, C + 1], FP, tag="ps_x")
            for k in range(8):
                mm(pe[:], ET[k][:, 128 * pt:128 * (pt + 1)], xv[b][:, 257 * k:257 * k + 257],
                   start=(k == 0), stop=(k == 7))
            negD = sm.tile([128, 1], FP, tag="negD")
            nc.vector.tensor_scalar_mul(negD[:], pe[:, C:C + 1], -1.0)
            nrd = sm.tile([128, 1], FP, tag="nrd")
            nc.vector.reciprocal(nrd[:], negD[:])
            spr = sm.tile([128, C], FP, tag="spr")
            nc.vector.scalar_tensor_tensor(spr[:], pe[:, 0:C], nrd[:, 0:1],
                                           xvv[b][:, pt, 0:C],
                                           op0=ALU.mult, op1=ALU.add)
            ptr = ps_x.tile([128, C], FP, tag="ps_x")
            for ch in range(2):
                nc.tensor.transpose(_fr(ptr[:, 128 * ch:128 * (ch + 1)]),
                                    _fr(spr[:, 128 * ch:128 * (ch + 1)]),
                                    _fr(ident[:, :]))
            nc.vector.tensor_copy(
                _fr(spTv[:, :, 128 * pt:128 * (pt + 1)]),
                ptr[:].rearrange("p (ch q) -> p ch q", ch=2))

        # out[o,:] = relu(Wlg @ spiral^T + fa (x) ea + x)
        xmv = xmat[b][:].rearrange("c (c2 q) -> c c2 q", c2=2)
        for ot in range(2):
            po = ps_w.tile([128, HW], FP, tag="ps_w")
            for nh in range(2):
                sl = slice(512 * nh, 512 * (nh + 1))
                for ct in range(2):
                    mm(po[:, sl], WlgT[ct][:, 128 * ot:128 * (ot + 1)],
                       spTv[:, ct, sl], start=(ct == 0), stop=False)
                mm(po[:, sl], fab[b][0:1, 128 * ot:128 * (ot + 1)],
                   ea[b][0:1, sl], start=False, stop=False)
                mm(po[:, sl], ident[:, :], xmv[:, ot, sl],
                   start=False, stop=True)
            ob = obp.tile([128, HW], FP, tag="ob")
            nc.scalar.activation(_fr(ob[:]), po[:], AF.Relu)
            nc.sync.dma_start(_fr(out_v[b][:, ot, :]), _fr(ob[:]))


# ---------------------------------------------------------------------------
# host driver
# ---------------------------------------------------------------------------

def _prep_shared(inputs):
    return {"wpackH": _pack_h(inputs), "wpackS": _pack_s(inputs)}


_NC_CACHE = {}


def kernel(**inputs):
    global LAST_EXEC_NS, LAST_RESULT
    if "nc" not in _NC_CACHE:
        _NC_CACHE["nc"] = _build_nc()
    nc = _NC_CACHE["nc"]

    x = np.ascontiguousarray(inputs["x"], dtype=np.float32)
    B = x.shape[0]
    shared = _prep_shared(inputs)
    in_maps = []
    for i in range(NCORES):
        m = dict(shared)
        m["x"] = np.ascontiguousarray(
            x[i * BPC:(i + 1) * BPC].reshape(BPC, C * HW))
        in_maps.append(m)

    trace = os.environ.get("KERNEL_TRACE", "0") == "1"
    res = run_bass_kernel_spmd(nc, in_maps, list(range(NCORES)), trace=trace)
    LAST_RESULT = res
    LAST_EXEC_NS = getattr(res, "exec_time_ns", None)

    out = np.empty((B, C, 32, 32), np.float32)
    for i in range(NCORES):
        out[i * BPC:(i + 1) * BPC] = res.results[i]["out"].reshape(BPC, C, 32, 32)
    return out
